# revision 32
# baseline (speedup 1.0000x reference)
"""Trainium2 Bass kernel for the YOLO-style grid loss (nn_Loss_12326556139840).

Strategy: data parallel over 8 NeuronCores with host-side obj/noobj cell
compaction (layout-only prep; all value arithmetic stays on device).

Observation: every loss term except the no-object confidence term is
masked by cell_obj; no-object cells (about half, since obj is a coin
flip) contribute ONLY 0.5*(pc0^2 + pc1^2).  So the host gathers the obj
cells into a dense plane-major fp16 layout (18 box planes + 40 class
planes per cell) and ships just the two predicted-conf planes for the
noobj cells.  This halves HBM traffic and removes every obj-mask
multiply from the device program (on the obj partition obj == 1).

Padding cells are synthesized to contribute exactly zero to all terms:
pre = [x0=1,y0=1,x1=0,y1=0, wh=0, c0=0,c1=1], tgt = 0.  (Both IoUs tie
-> r=n=0 -> resp=0, nonresp=0; fxy(0)=1 so the xy residual of box 0 is
1-1=0; conf targets are rn=0 for the resp slot and 1-rn=1 for the
non-resp slot, matching c0=0, c1=1.)

Device-side math per obj cell (branchless, all fp16 unit-stride):
  - iw = relu(min(pw+tw-2|px-tx|, 2*min(pw,tw)))  (2x-scaled overlap)
  - responsible box via cross-multiplied IoU compare (i1*A0 vs i0*A1;
    A = sum of areas; the i0*i1 union terms cancel; +4e-4 keeps the
    reference's eps tie-break)
  - fxy = frac(7*xy_nr) with frac<=0 -> 1, via fp16 +1032 rounding
  - conf: dc = pc0 + q and dna = pc1 - q - 1 with q = r*(pc1-pc0-1)-n
  - residual planes masked by [b==resp] only; loss weights are folded
    into the ACT Square scale (sqrt5 for xy/wh, sqrt.5 for no-conf).

Engine split: DVE does ALL the algebra (2x/4x perf modes; one group of
C=396 cells per partition so per-instruction fixed costs ~150ns are paid
once; GPSIMD is unused — it runs these ops 2-4x slower AND taxes
concurrent DVE ~1.5-2x via SBUF contention), ACT does abs/sqrt and all
Square+accum reductions with the loss weights folded into the Square
scale.  The class planes arrive as two [10 pre | 10 tgt] chunks so
diff+square can start after half the class DMA; residuals are computed
box0-first so box0's square overlaps box1's masking, and the ACT queue
is emitted in dependency-readiness order (it executes in order, so one
late-dependency square would head-of-line-block the rest).  Output:
[128, 7] fp32 accumulator columns per core; the host sums and divides
by B.
"""

import numpy as np

import concourse.bacc as bacc
import concourse.tile as tile
from concourse import mybir
from concourse.bass_utils import run_bass_kernel_spmd

F32 = mybir.dt.float32
F16 = mybir.dt.float16
Alu = mybir.AluOpType
Act = mybir.ActivationFunctionType

B = 16384
NCORES = 8
NCELL = B * 49               # 802816 cells total
P = 128
C = 396                      # obj cells per partition (50,171 actual + pad)
CAP = P * C                  # 52224 obj-cell capacity per core
CN = C                       # noobj cells per partition
CAPN = P * CN                # 52224 noobj-cell capacity per core

EPS = 1e-7
SQRT5 = float(np.sqrt(5.0))
SQRTH = float(np.sqrt(0.5))

# box-plane channel picks from concat(pre, tgt) [.., 60]:
#   pxy4 (x0,y0,x1,y1) | txy4 | pwh4 (w0,h0,w1,h1) | twh4 | pc2
BOX_CH = [0, 1, 5, 6, 30, 31, 35, 36,
          2, 3, 7, 8, 32, 33, 37, 38, 4, 9]
# class channels in two [10 pre | 10 tgt] chunks
CLS_CH = (list(range(10, 20)) + list(range(40, 50))
          + list(range(20, 30)) + list(range(50, 60)))
# pad cell: zero contribution to every loss term (see module docstring)
PAD_ROW = np.array(
    [1, 1, 0, 0, 0, 0, 0, 0, 0, 0, 0, 0, 0, 0, 0, 0, 0, 1],
    dtype=np.float16,
)


def _build():
    nc = bacc.Bacc()
    box_d = nc.declare_dram_parameter("box", [P, 18 * C], F16, isOutput=False)
    cls_d = nc.declare_dram_parameter("cls", [2, P, 20 * C], F16, isOutput=False)
    nob_d = nc.declare_dram_parameter("nob", [P, 2 * CN], F16, isOutput=False)
    out_d = nc.declare_dram_parameter("out", [P, 7], F32, isOutput=True)

    with tile.TileContext(nc) as tc:
        with (
            tc.tile_pool(name="bx", bufs=1) as bxp,
            tc.tile_pool(name="kl", bufs=1) as klp,
            tc.tile_pool(name="rr", bufs=1) as rrp,
            tc.tile_pool(name="w4", bufs=1) as w4,
            tc.tile_pool(name="w2", bufs=1) as w2,
            tc.tile_pool(name="w1", bufs=1) as w1,
            tc.tile_pool(name="one", bufs=1) as one,
        ):
            v = nc.vector
            s = nc.scalar
            g_ = nc.gpsimd

            acc = one.tile([P, 7], F32, tag="acc")
            v.memset(acc, 0.0)
            eps_b = one.tile([P, 1], F32, tag="eps")
            v.memset(eps_b, EPS)

            # ---- input DMAs: spread across engine queues so the transfers
            # run on different DMA engines in parallel (one queue serializes)
            bx = bxp.tile([P, 18, C], F16, tag="bx")
            kl = klp.tile([P, 40, C], F16, tag="kl")
            box_v = box_d[:].rearrange("p (q c) -> p q c", c=C)
            nc.sync.dma_start(out=bx[:, 8:16, :], in_=box_v[:, 8:16, :])
            nc.sync.dma_start(out=bx[:, 0:8, :], in_=box_v[:, 0:8, :])
            nc.sync.dma_start(
                out=kl[:, 0:20, :],
                in_=cls_d[0].rearrange("p (q c) -> p q c", c=C),
            )
            nc.sync.dma_start(out=bx[:, 16:18, :], in_=box_v[:, 16:18, :])
            nc.sync.dma_start(
                out=kl[:, 20:40, :],
                in_=cls_d[1].rearrange("p (q c) -> p q c", c=C),
            )
            nob = one.tile([P, 2, CN], F16, tag="nob")
            nc.sync.dma_start(
                out=nob, in_=nob_d[:].rearrange("p (q c) -> p q c", c=CN)
            )

            pxy4 = bx[:, 0:4, :]
            txy4 = bx[:, 4:8, :]
            pwh4 = bx[:, 8:12, :]
            twh4 = bx[:, 12:16, :]
            pc2 = bx[:, 16:18, :]
            R = rrp.tile([P, 30, C], F16, tag="R")

            # ---------------- IoU -> r, n, rn ----------------
            # wh-only ops first: their DMA lands first so DVE starts on
            # them while the xy planes stream in
            t_d = w4.tile([P, 4, C], F16, tag="d")
            t_s = w4.tile([P, 4, C], F16, tag="s")
            t_m = w4.tile([P, 4, C], F16, tag="m")
            v.tensor_add(t_s, pwh4, twh4)                       # s4
            v.tensor_tensor(t_m, pwh4, twh4, op=Alu.min)        # m4
            t_ap = w2.tile([P, 2, C], F16, tag="ap")
            t_at = w2.tile([P, 2, C], F16, tag="at")
            v.tensor_mul(t_ap, pwh4[:, 0::2, :], pwh4[:, 1::2, :])
            v.tensor_mul(t_at, twh4[:, 0::2, :], twh4[:, 1::2, :])
            v.tensor_add(t_ap, t_ap, t_at)                      # A2
            v.tensor_scalar_mul(t_m, t_m, 2.0)                  # mm4
            t_sp = w4.tile([P, 4, C], F16, tag="sp")
            t_st = w4.tile([P, 4, C], F16, tag="st")
            s.activation(t_sp, pwh4, Act.Sqrt, bias=eps_b)
            s.activation(t_st, twh4, Act.Sqrt, bias=eps_b)
            v.tensor_sub(t_d, pxy4, txy4)
            t_d2 = w2.tile([P, 2, C], F16, tag="d2")
            v.tensor_copy(t_d2, txy4[:, 0:2, :])                # xy_nr default
            s.activation(t_d, t_d, Act.Abs, scale=2.0)          # a4 = 2|d|
            v.tensor_sub(t_s, t_s, t_d)                         # e4
            v.tensor_tensor(t_s, t_m, t_s, op=Alu.min)          # iw4
            v.tensor_single_scalar(t_s, t_s, 0.0, op=Alu.max)
            t_i = w2.tile([P, 2, C], F16, tag="i")
            v.tensor_mul(t_i, t_s[:, 0::2, :], t_s[:, 1::2, :])  # i2
            v.tensor_scalar_add(t_i, t_i, 4e-4)
            t_cr = w2.tile([P, 2, C], F16, tag="cr")
            v.tensor_mul(t_cr, t_i, t_ap[:, ::-1, :])
            r_ = w1.tile([P, C], F16, tag="r")
            n_ = w1.tile([P, C], F16, tag="n")
            n_i = w1.tile([P, C], mybir.dt.int16, tag="ni")
            v.tensor_tensor(r_, t_cr[:, 1, :], t_cr[:, 0, :], op=Alu.is_gt)
            v.tensor_tensor(n_, t_cr[:, 0, :], t_cr[:, 1, :], op=Alu.is_gt)
            v.tensor_tensor(n_i, t_cr[:, 0, :], t_cr[:, 1, :], op=Alu.is_gt)

            # ---- conf targets: dc = pc0 + q, dna+1 = pc1 - q ----
            # (on DVE: GPSIMD runs these 2-4x slower AND taxes concurrent
            # DVE ops ~1.5-2x via SBUF contention, so it nets negative)
            dpc = w1.tile([P, C], F16, tag="dpc")
            v.tensor_sub(dpc, pc2[:, 1, :], pc2[:, 0, :])
            v.tensor_scalar_add(dpc, dpc, -1.0)
            v.tensor_mul(dpc, dpc, r_)
            v.tensor_sub(dpc, dpc, n_)                          # q = r*(dpc-1)-n
            v.tensor_add(R[:, 8, :], pc2[:, 0, :], dpc)         # dc
            v.tensor_sub(R[:, 9, :], pc2[:, 1, :], dpc)         # dna + 1
            v.tensor_scalar(R[:, 9, :], R[:, 9, :], 1.0, SQRTH,
                            op0=Alu.subtract, op1=Alu.mult)

            # ---------------- class diffs (two chunks) ----------------
            v.tensor_sub(R[:, 10:20, :], kl[:, 0:10, :], kl[:, 10:20, :])
            v.tensor_sub(R[:, 20:30, :], kl[:, 20:30, :], kl[:, 30:40, :])

            # ---------------- xy_nr blend + fxy ----------------
            ni_b2 = n_i.unsqueeze(1).broadcast_to([P, 2, C])
            v.copy_predicated(t_d2, ni_b2, txy4[:, 2:4, :])     # xy_nr
            t_u = w2.tile([P, 2, C], F16, tag="u")
            t_rt = w2.tile([P, 2, C], F16, tag="rt")
            v.tensor_scalar_mul(t_u, t_d2, 7.0)
            # (u - 0.5005) + 1032 lands in [1024, 2048) where fp16 grain
            # is exactly 1.0 -> the fp16 store rounds to an integer.
            v.tensor_scalar(t_rt, t_u, 0.5005, 1032.0,
                            op0=Alu.subtract, op1=Alu.add)
            v.tensor_scalar(t_rt, t_rt, 1032.0, None, op0=Alu.subtract)
            v.tensor_sub(t_u, t_u, t_rt)                        # fxy

            # ---------------- box residuals + masks ----------------
            # masks with sqrt(5) folded in: box planes join the weight-1
            # square; dc complete in R8; R9 holds dna+1 -> sqrt(.5)*(R9-1)
            m0 = w1.tile([P, C], F16, tag="m0")
            v.tensor_scalar(m0, r_, -SQRT5, SQRT5, op0=Alu.mult, op1=Alu.add)
            v.tensor_scalar_mul(r_, r_, SQRT5)
            # box-split residuals: box0 resids+mask complete first so its
            # square overlaps box1's resids; only box1's tail is C-halved
            H = C // 2
            m0_b4 = m0.unsqueeze(1).broadcast_to([P, 4, C])
            r_b4 = r_.unsqueeze(1).broadcast_to([P, 4, C])
            v.tensor_sub(R[:, 0:2, :], pxy4[:, 0:2, :], t_u)
            v.tensor_sub(R[:, 2:4, :], t_sp[:, 0:2, :], t_st[:, 0:2, :])
            v.tensor_mul(R[:, 0:4, :], R[:, 0:4, :], m0_b4)
            v.tensor_sub(R[:, 4:6, :], pxy4[:, 2:4, :], t_u)
            v.tensor_sub(R[:, 6:8, :], t_sp[:, 2:4, :], t_st[:, 2:4, :])
            v.tensor_mul(R[:, 4:8, 0:H], R[:, 4:8, 0:H], r_b4[:, :, 0:H])
            v.tensor_mul(R[:, 4:8, H:C], R[:, 4:8, H:C], r_b4[:, :, H:C])

            # no-object cells: 0.5 * sum(pc^2)  (independent, fills ACT idle)
            s.activation(nob, nob, Act.Square, scale=SQRTH,
                         accum_out=acc[:, 6:7])

            # ---------------- square + accumulate (ACT) ----------------
            s.activation(R[:, 10:20, :], R[:, 10:20, :], Act.Square,
                         accum_out=acc[:, 4:5])
            s.activation(R[:, 20:30, :], R[:, 20:30, :], Act.Square,
                         accum_out=acc[:, 5:6])
            s.activation(R[:, 8:10, :], R[:, 8:10, :], Act.Square,
                         accum_out=acc[:, 3:4])
            s.activation(R[:, 0:4, :], R[:, 0:4, :], Act.Square,
                         accum_out=acc[:, 0:1])
            s.activation(R[:, 4:8, 0:H], R[:, 4:8, 0:H], Act.Square,
                         accum_out=acc[:, 1:2])
            s.activation(R[:, 4:8, H:C], R[:, 4:8, H:C], Act.Square,
                         accum_out=acc[:, 2:3])

            nc.sync.dma_start(out=out_d[:], in_=acc[:])

    nc.compile()
    return nc


def _prep_core(all16: np.ndarray, obj_idx, non_idx, core: int):
    """all16: fp16 [NCELL, 60] = concat(pre, tgt) flattened per cell."""
    qo = (len(obj_idx) + NCORES - 1) // NCORES
    qn = (len(non_idx) + NCORES - 1) // NCORES
    assert qo <= CAP and qn <= CAPN, (qo, qn)
    oi = obj_idx[core * qo : (core + 1) * qo]
    ni = non_idx[core * qn : (core + 1) * qn]

    gob = all16[oi]
    box = np.empty((CAP, 18), dtype=np.float16)
    box[:] = PAD_ROW
    box[: len(oi)] = gob[:, BOX_CH]
    cls = np.zeros((CAP, 40), dtype=np.float16)
    cls[: len(oi)] = gob[:, CLS_CH]
    box = np.ascontiguousarray(
        box.reshape(P, C, 18).transpose(0, 2, 1)
    ).reshape(P, 18 * C)
    cls = np.ascontiguousarray(
        cls.reshape(P, C, 2, 20).transpose(2, 0, 3, 1)
    ).reshape(2, P, 20 * C)

    nob = np.zeros((CAPN, 2), dtype=np.float16)
    nob[: len(ni)] = all16[ni][:, [4, 9]]
    nob = np.ascontiguousarray(
        nob.reshape(P, CN, 2).transpose(0, 2, 1)
    ).reshape(P, 2 * CN)
    return {"box": box, "cls": cls, "nob": nob}


_NC_CACHE = None


def kernel(pre: np.ndarray, target: np.ndarray) -> np.ndarray:
    global _NC_CACHE
    if _NC_CACHE is None:
        _NC_CACHE = _build()
    nc = _NC_CACHE

    pre3 = np.asarray(pre, dtype=np.float32).reshape(NCELL, 30)
    tgt3 = np.asarray(target, dtype=np.float32).reshape(NCELL, 30)
    objmask = tgt3[:, 4] > 0
    all16 = np.concatenate(
        [pre3.astype(np.float16), tgt3.astype(np.float16)], axis=1
    )
    obj_idx = np.flatnonzero(objmask)
    non_idx = np.flatnonzero(~objmask)

    in_maps = [
        _prep_core(all16, obj_idx, non_idx, i) for i in range(NCORES)
    ]
    res = run_bass_kernel_spmd(nc, in_maps, core_ids=list(range(NCORES)))
    total = 0.0
    for r in res.results:
        total += float(np.sum(r["out"].astype(np.float64)))
    return np.float32(total / B)
